# revision 12
# baseline (speedup 1.0000x reference)
"""DendriticMLP Trainium2 kernel — 8-core batch-data-parallel.

v4: fp16 "hi" main pass + single fp8-e4m3 DoubleRow matmul fusing both
hi*lo cross terms (lo parts pre-scaled by 2^12), accumulating into one
fp32 PSUM at 2^12 scale (consumers fold 2^-12 back for free). On top of
v2:
  - layer-1 top-k bisection runs entirely on ACT (Sign+accum counts)
    and Pool (scalar updates), interleaved into the dend2 chunk loop,
    so DVE stays dedicated to the dend max/min PSUM reduces and the PE
    never stalls on PSUM-bank recycling.
  - bisection searches [0, M] (k-th winner is always positive here)
    instead of [-2M, 2M]: two bits tighter window, measurably lower
    top-k membership error.
  - the output-layer loop runs on b-tile PAIRS so w_out streams twice
    instead of four times (-16.8 MB DMA/core), and the two tail
    bisections of a pair run concurrently on DVE and ACT.
  - all weight/segment/stationary DRAM layouts are pre-packed
    partition-major on the host so every DMA line is 4-8 KB contiguous
    per partition (the v2 `rearrange` loads moved 1 KB lines).

Pipeline per core (B_local=512 rows, 4 b-tiles):
  y1 = x @ w1.T + b1
  dend1 = ctx @ seg1_flat.T (41 chunks) -> strided max/min reduce over
      10 segments (DVE) -> sel = where(mx+mn>0, mx, mn) ->
      gate = sigmoid(2^-12 sel) (ACT)
  g = y1*gate; exact top-k (k=102) via 20-iteration count bisection
  h split to fp16 hi (+2^12-scaled lo), transposed on PE, fp8 recast
  layer 2 same; out = h2 @ w_out.T + b_out.
"""
import numpy as np
from contextlib import ExitStack

import concourse.bass as bass
import concourse.tile as tile
from concourse import bacc, mybir, masks
from concourse.bass_utils import run_bass_kernel_spmd
import ml_dtypes

F32 = mybir.dt.float32
F16 = mybir.dt.float16
F8 = mybir.dt.float8e4
AF = mybir.ActivationFunctionType
OP = mybir.AluOpType
AX = mybir.AxisListType
PM = mybir.MatmulPerfMode

# problem dims (hardcoded per contract)
B, D_IN, H, S, D_CTX, D_OUT = 4096, 1024, 2048, 10, 1024, 1024
KW = 102                 # k-winners per row
NCORES = 8
BL = B // NCORES         # 512 rows per core
BT = BL // 128           # 4 b-tiles of 128 rows
NITER = 20               # bisection iterations

HS = H * S               # 20480
CHW = 510                # dend chunk width (51 groups of 10)
NCH = HS // CHW          # 40 full chunks
TAIL = HS - NCH * CHW    # 80 (8 groups)
GR = CHW // S            # 51 groups per chunk
KT_IN = D_IN // 128      # 8 contraction tiles for d=1024
KT_H = H // 128          # 16 contraction tiles for d=2048

SCL = 4096.0             # 2^12 psum scale
ISCL = 1.0 / SCL


def build_kernel(loop_n=None, weights_internal=False):
    """weights_internal=True: big replicated weights become Internal DRAM
    (uninitialized) so timing runs skip the host transfer; the on-device
    DMA and compute per iteration are identical."""
    nc = bacc.Bacc("TRN2", target_bir_lowering=False, debug=False,
                   num_devices=NCORES)

    def din(name, shape, dt=F32):
        return nc.dram_tensor(name, shape, dt, kind="ExternalInput").ap()

    if weights_internal:
        _din_small = din

        def din(name, shape, dt=F32):  # noqa: F811
            if name in ("x1s", "x8", "c1s", "c8"):
                return _din_small(name, shape, dt)
            return nc.dram_tensor(name, shape, dt, kind="Internal").ap()

    # activations, pre-transposed partition-major on the host:
    # fp16 hi (stationary side pre-scaled 2^12) + fp8 plane pairs
    x1s_d = din("x1s", [128, KT_IN * BL], F16)
    x8_d = din("x8", [128, 2, KT_IN * BL], F8)     # planes (x2*S, x1)
    c1s_d = din("c1s", [128, KT_IN * BL], F16)
    c8_d = din("c8", [128, 2, KT_IN * BL], F8)
    # dense-layer weights: [chunk, qgroup, 128, 4(k), ...] contiguous lines
    w1t_d = din("w1t", [H // 512, KT_IN // 4, 128, 4, 512], F16)
    w1t8_d = din("w1t8", [H // 512, KT_IN // 4, 128, 4, 2, 512], F8)
    w2t_d = din("w2t", [H // 512, KT_H // 4, 128, 4, 512], F16)
    w2t8_d = din("w2t8", [H // 512, KT_H // 4, 128, 4, 2, 512], F8)
    wot_d = din("wot", [D_OUT // 512, KT_H // 4, 128, 4, 512], F16)
    wot8_d = din("wot8", [D_OUT // 512, KT_H // 4, 128, 4, 2, 512], F8)
    # dendrite segments, 41 uniform padded chunks, partition-major
    sg1_d = din("sg1", [NCH + 1, 128, KT_IN, 512], F16)
    sg18_d = din("sg18", [NCH + 1, 128, KT_IN, 2, 512], F8)
    sg2_d = din("sg2", [NCH + 1, 128, KT_IN, 512], F16)
    sg28_d = din("sg28", [NCH + 1, 128, KT_IN, 2, 512], F8)
    # biases pre-scaled by 2^12, fp16 hi/lo rows
    b1d = din("b1d", [2, H], F16)
    b2d = din("b2d", [2, H], F16)
    bod = din("bod", [2, D_OUT], F16)
    out_d = nc.dram_tensor("out", [BL, D_OUT], F32, kind="ExternalOutput").ap()

    with tile.TileContext(nc) as tc, ExitStack() as ctx:
        if loop_n is not None:
            ctx.enter_context(tc.For_i(0, loop_n, 1))
        cpool = ctx.enter_context(tc.tile_pool(name="const", bufs=1))
        apool = ctx.enter_context(tc.tile_pool(name="acts", bufs=1))
        ypool = ctx.enter_context(tc.tile_pool(name="y", bufs=BT))
        selpool = ctx.enter_context(tc.tile_pool(name="sel", bufs=BT))
        mnpool = ctx.enter_context(tc.tile_pool(name="mn", bufs=2))
        wpool = ctx.enter_context(tc.tile_pool(name="w", bufs=4))
        spool = ctx.enter_context(tc.tile_pool(name="seg", bufs=2))
        outpool = ctx.enter_context(tc.tile_pool(name="osb", bufs=2))
        tinypool = ctx.enter_context(tc.tile_pool(name="tiny", bufs=1))
        hpool = ctx.enter_context(tc.tile_pool(name="hsplit", bufs=1))
        psy = ctx.enter_context(tc.tile_pool(name="psy", bufs=BT, space="PSUM"))
        psd = ctx.enter_context(tc.tile_pool(name="psd", bufs=2, space="PSUM"))
        pst = ctx.enter_context(tc.tile_pool(name="pst", bufs=2, space="PSUM"))

        # constants
        ident16 = cpool.tile([128, 128], F16)
        masks.make_identity(nc, ident16[:])
        ones = cpool.tile([2, 128], F16)
        nc.gpsimd.memset(ones[:], 1.0)
        cnt_bias = cpool.tile([128, 1], F32)
        nc.gpsimd.memset(cnt_bias[:], float(H - 2 * KW) + 0.5)
        bpool = ctx.enter_context(tc.tile_pool(name="bias", bufs=2))

        # ctx stationary (lives through both dend phases)
        c1s_sb = apool.tile([128, KT_IN * BL], F16)
        c8_sb = apool.tile([128, 2, KT_IN * BL], F8)
        nc.sync.dma_start(c1s_sb[:], c1s_d)
        nc.scalar.dma_start(c8_sb[:], c8_d)
        # x stationary (dead after y1) shares slots with the h pair below
        x1s_sb = apool.tile([128, KT_H * 512], F16, tag="stat16", name="x1s_sb")
        x8_sb = apool.tile([128, 2, KT_H * 512], F8, tag="stat8", name="x8_sb")
        nc.sync.dma_start(x1s_sb[:, :KT_IN * BL], x1s_d)
        nc.scalar.dma_start(x8_sb[:, :, :KT_IN * BL], x8_d)

        def st16(sb, kk, bt):
            return sb[:, kk * BL + bt * 128: kk * BL + (bt + 1) * 128]

        def st8(sb, kk, bt):
            return sb[:, :, kk * BL + bt * 128: kk * BL + (bt + 1) * 128]

        # per-bt tiny state: cols 0=M 1=lo 2=w 3=t 4=pred 5=cnt 6=tneg
        tiny = [tinypool.tile([128, 8], F32, tag=f"tiny{bt}", name=f"tiny{bt}")
                for bt in range(BT)]

        scr_cell = []

        def get_scr():
            if not scr_cell:
                scr_cell.append(apool.tile([128, H], F16, tag="scr",
                                           name="scr"))
            return scr_cell[0]

        act_scr = apool.tile([128, H], F16, tag="ascr", name="act_scr")

        def yphase(s16, s8, kt, w16_dram, w8_dram, bias_dram, width, lay,
                   bts=None, ytiles=None):
            """Dense y = act @ W.T (+bias), planB, for the given b-tiles."""
            if bts is None:
                bts = tuple(range(BT))
            if ytiles is None:
                ytiles = [ypool.tile([128, H], F32, tag="y",
                                     name=f"y{lay}_{i}") for i in range(BT)]
            nch = width // 512
            for n in range(nch):
                ps = {bt: psy.tile([128, 512], F32, tag="psy",
                                   name=f"psy{bt}") for bt in bts}
                for q in range(kt // 4):
                    wt = wpool.tile([128, 4, 512], F16, tag="w")
                    nc.sync.dma_start(wt[:], w16_dram[n, q])
                    wt8 = wpool.tile([128, 4, 2, 512], F8, tag="w8")
                    nc.scalar.dma_start(wt8[:], w8_dram[n, q])
                    for kk in range(4):
                        k = q * 4 + kk
                        for bt in bts:
                            nc.tensor.matmul(ps[bt][:], s16(k, bt),
                                             wt[:, kk, :],
                                             start=(k == 0), stop=False)
                            nc.tensor.matmul(ps[bt][:], s8(k, bt),
                                             wt8[:, kk, :, :],
                                             start=False, stop=False,
                                             perf_mode=PM.DoubleRow)
                bsb = bpool.tile([2, 512], F16, tag="bias", name="bsb")
                nc.sync.dma_start(bsb[:], bias_dram[:, n * 512:(n + 1) * 512])
                for bt in bts:
                    nc.tensor.matmul(ps[bt][:], ones[:], bsb[:],
                                     start=False, stop=True)
                for bt in bts:
                    nc.scalar.activation(ytiles[bt][:, n * 512:(n + 1) * 512],
                                         ps[bt][:], AF.Copy, scale=ISCL)
            return ytiles

        def dendphase(sg16, sg8, seltiles, filler=None):
            """dend matmuls + per-chunk absmax-signed sel on DVE:
            sel-slice = where(mx+mn>0, mx, mn) right after the max/min
            reduces. filler() interleaves ACT/Pool bisection steps."""
            for c in range(NCH + 1):
                w = CHW if c < NCH else TAIL
                g = GR if c < NCH else TAIL // S
                sgt = spool.tile([128, KT_IN, 512], F16, tag="seg", name="sgt")
                nc.sync.dma_start(sgt[:], sg16[c])
                sgt8 = spool.tile([128, KT_IN, 2, 512], F8, tag="seg8",
                                  name="sgt8")
                nc.scalar.dma_start(sgt8[:], sg8[c])
                for bt in range(BT):
                    pd = psd.tile([128, 512], F32, tag="psd")
                    for k in range(KT_IN):
                        nc.tensor.matmul(pd[:, :w], st16(c1s_sb, k, bt),
                                         sgt[:, k, :w],
                                         start=(k == 0), stop=False)
                    for k in range(KT_IN):
                        nc.tensor.matmul(pd[:, :w], st8(c8_sb, k, bt),
                                         sgt8[:, k, :, :w],
                                         start=False, stop=(k == KT_IN - 1),
                                         perf_mode=PM.DoubleRow)
                    view = pd[:, :w].rearrange("p (g s) -> p g s", s=S)
                    sl = seltiles[bt][:, c * GR:c * GR + g]
                    mnt = mnpool.tile([128, 64], F32, tag="mnt", name="mnt")
                    prt = mnpool.tile([128, 64], F32, tag="prt", name="prt")
                    nc.vector.tensor_reduce(sl, view, axis=AX.X, op=OP.max)
                    nc.vector.tensor_reduce(mnt[:, :g], view, axis=AX.X,
                                            op=OP.min)
                    nc.vector.tensor_add(prt[:, :g], sl, mnt[:, :g])
                    nc.vector.tensor_scalar(prt[:, :g], prt[:, :g], 0.0, None,
                                            op0=OP.is_le)
                    nc.vector.copy_predicated(
                        sl, prt[:, :g].bitcast(mybir.dt.int32), mnt[:, :g])
                if filler is not None:
                    filler(3)

        def selgate(seltiles):
            """gate = sigmoid(2^-12 * sel) in place."""
            for bt in range(BT):
                nc.scalar.activation(seltiles[bt][:], seltiles[bt][:],
                                     AF.Sigmoid, scale=ISCL)

        def gate_mul(ytiles, seltiles, bts=None):
            """g = y*gate in place on y tile; M = absmax(g) into tiny col 0."""
            for bt in (range(BT) if bts is None else bts):
                nc.vector.tensor_mul(ytiles[bt][:], ytiles[bt][:],
                                     seltiles[bt][:])
                nc.vector.tensor_reduce(tiny[bt][:, 0:1], ytiles[bt][:],
                                        axis=AX.X, op=OP.max,
                                        apply_absolute_value=True)

        def bisect_steps(ytile, bt):
            """Generator: one bisection iteration per yield, ENTIRELY on ACT
            (midpoint form): probe mu, half-step s; mu += sign*s each iter.
            Only the absmax init (DVE) and final mask (DVE) cross engines,
            so dend reduces never wait behind bisection.
            Search [0, 1.001*M]: the 102nd of 2048 is always > 0 here."""
            t = tiny[bt]
            M, mu, sp, tt_, sg, cnt = (t[:, i:i + 1] for i in range(6))
            tneg = t[:, 6:7]
            nc.scalar.activation(mu, M, AF.Copy, scale=0.5005, bias=2.5e-31)
            nc.scalar.activation(sp, M, AF.Copy, scale=0.25025,
                                 bias=1.25e-31)
            yield
            for it in range(NITER):
                nc.scalar.activation(tneg, mu, AF.Copy, scale=-1.0)
                # sum(sign(g - mu)); count>=KW  <=>  sum >= 2*KW - H
                nc.scalar.activation(act_scr[:], ytile[:], AF.Sign,
                                     bias=tneg, accum_out=cnt)
                nc.scalar.activation(sg, cnt, AF.Sign, bias=cnt_bias[:])
                nc.scalar.activation(mu, sg, AF.Identity, scale=sp, bias=mu)
                nc.scalar.activation(sp, sp, AF.Copy, scale=0.5)
                yield
            # lower bound of final bracket: lo = mu - 2*s
            nc.scalar.activation(tt_, sp, AF.Identity, scale=-2.0, bias=mu)
            nc.vector.scalar_tensor_tensor(ytile[:], ytile[:], tt_, ytile[:],
                                           op0=OP.is_ge, op1=OP.mult)

        def make_filler(gens):
            """Round-robin advance the chain generators n steps per call."""
            alive = list(gens)

            def fill(n):
                for _ in range(n):
                    if not alive:
                        return
                    g = alive.pop(0)
                    try:
                        next(g)
                        alive.append(g)
                    except StopIteration:
                        pass
            fill.drain = lambda: fill(10**6)
            return fill

        def bisect_chain(ytile, bt, on_act):
            """One b-tile's top-k bisection + in-place mask. The whole chain
            runs on ONE engine (ACT midpoint form, or DVE lo/w form) so no
            per-iteration cross-engine semaphores exist."""
            t = tiny[bt]
            M, lo, w, tt_, pred, cnt = (t[:, i:i + 1] for i in range(6))
            tneg = t[:, 6:7]
            if on_act:
                mu, sp, sg = lo, w, pred
                nc.scalar.activation(mu, M, AF.Copy, scale=0.5005,
                                     bias=2.5e-31)
                nc.scalar.activation(sp, M, AF.Copy, scale=0.25025,
                                     bias=1.25e-31)
                for it in range(NITER):
                    nc.scalar.activation(tneg, mu, AF.Copy, scale=-1.0)
                    nc.scalar.activation(act_scr[:], ytile[:], AF.Sign,
                                         bias=tneg, accum_out=cnt)
                    nc.scalar.activation(sg, cnt, AF.Sign, bias=cnt_bias[:])
                    nc.scalar.activation(mu, sg, AF.Identity, scale=sp,
                                         bias=mu)
                    nc.scalar.activation(sp, sp, AF.Copy, scale=0.5)
                nc.scalar.activation(tt_, sp, AF.Identity, scale=-2.0,
                                     bias=mu)
            else:
                ve = nc.vector
                ve.tensor_scalar(w, M, 1.001, 1e-30, op0=OP.mult, op1=OP.add)
                ve.tensor_scalar_mul(lo, M, 0.0)
                for it in range(NITER):
                    ve.tensor_scalar_mul(w, w, 0.5)
                    ve.tensor_add(tt_, lo, w)
                    ve.scalar_tensor_tensor(
                        get_scr()[:], ytile[:], tt_, ytile[:],
                        op0=OP.is_ge, op1=OP.bypass, accum_out=cnt)
                    ve.tensor_scalar(pred, cnt, float(KW), None,
                                     op0=OP.is_ge)
                    ve.scalar_tensor_tensor(lo, pred, w, lo,
                                            op0=OP.mult, op1=OP.add)
                tt_ = lo
            nc.vector.scalar_tensor_tensor(ytile[:], ytile[:], tt_, ytile[:],
                                           op0=OP.is_ge, op1=OP.mult)

        # h split + transpose: produce fp16-scaled hi stationary + fp8 pair
        h1T = apool.tile([128, KT_H * 512], F16, tag="stat16", name="h1T")
        h8T = apool.tile([128, 2, KT_H * 512], F8, tag="stat8", name="h8T")

        def transpose_bt(ytile, bt):
            """ytile holds masked h (fp32). Writes h1T/h8T bt-columns."""
            h1 = hpool.tile([128, H], F16, tag="h1")
            nc.scalar.activation(h1[:], ytile[:], AF.Copy)
            h2s = hpool.tile([128, H], F16, tag="h2s")
            nc.vector.scalar_tensor_tensor(get_scr()[:], h1[:], -1.0,
                                           ytile[:], op0=OP.mult, op1=OP.add)
            nc.vector.tensor_scalar_mul(h2s[:], get_scr()[:], SCL)
            for kb in range(KT_H):
                dst = slice(kb * 512 + bt * 128, kb * 512 + (bt + 1) * 128)
                p1 = pst.tile([128, 128], F16, tag="pst", name="p1")
                nc.tensor.transpose(p1[:], h1[:, kb * 128:(kb + 1) * 128],
                                    ident16[:])
                nc.scalar.activation(h1T[:, dst], p1[:], AF.Copy, scale=SCL)
                nc.scalar.activation(h8T[:, 1, dst], p1[:], AF.Copy)
                p2 = pst.tile([128, 128], F16, tag="pst", name="p2")
                nc.tensor.transpose(p2[:], h2s[:, kb * 128:(kb + 1) * 128],
                                    ident16[:])
                nc.scalar.activation(h8T[:, 0, dst], p2[:], AF.Copy)

        def h_st16(k, bt):
            return h1T[:, k * 512 + bt * 128: k * 512 + (bt + 1) * 128]

        def h_st8(k, bt):
            return h8T[:, :, k * 512 + bt * 128: k * 512 + (bt + 1) * 128]

        # ---------------- layer 1 ----------------
        sel1 = [selpool.tile([128, H], F32, tag="sel", name=f"sel1_{i}")
                for i in range(BT)]
        y1 = yphase(lambda k, bt: st16(x1s_sb, k, bt),
                    lambda k, bt: st8(x8_sb, k, bt),
                    KT_IN, w1t_d, w1t8_d, b1d, H, lay=0)
        dendphase(sg1_d, sg18_d, sel1)
        selgate(sel1)
        gate_mul(y1, sel1)

        # dend2 issued with bisect1 interleaved as ACT/Pool filler steps:
        # DVE does only dend reduces, PE never waits on bisect1.
        sel2 = [selpool.tile([128, H], F32, tag="sel", name=f"sel2_{i}")
                for i in range(BT)]
        fill1 = make_filler([bisect_steps(y1[bt], bt) for bt in range(BT)])
        dendphase(sg2_d, sg28_d, sel2, filler=fill1)
        fill1.drain()
        selgate(sel2)

        for bt in range(BT):
            transpose_bt(y1[bt], bt)

        # ---------------- layer 2 ----------------
        y2 = yphase(h_st16, h_st8, KT_H, w2t_d, w2t8_d, b2d, H, lay=1)
        gate_mul(y2, sel2)

        # tail on b-tile pairs: both bisections of a pair run concurrently
        # (one on DVE, one on ACT), then the pair shares one w_out stream
        # (2x instead of 4x w_out DMA).
        for pr in range(BT // 2):
            bts = (2 * pr, 2 * pr + 1)
            for bt in bts:
                bisect_chain(y2[bt], bt, on_act=bool(bt % 2))
            for bt in bts:
                transpose_bt(y2[bt], bt)
            for n in range(D_OUT // 512):
                ps1 = {bt: psy.tile([128, 512], F32, tag="psy",
                                    name=f"pso{bt}") for bt in bts}
                for q in range(KT_H // 4):
                    wt = wpool.tile([128, 4, 512], F16, tag="w")
                    nc.sync.dma_start(wt[:], wot_d[n, q])
                    wt8 = wpool.tile([128, 4, 2, 512], F8, tag="w8")
                    nc.scalar.dma_start(wt8[:], wot8_d[n, q])
                    for kk in range(4):
                        k = q * 4 + kk
                        for bt in bts:
                            nc.tensor.matmul(ps1[bt][:], h_st16(k, bt),
                                             wt[:, kk, :],
                                             start=(k == 0), stop=False)
                            nc.tensor.matmul(ps1[bt][:], h_st8(k, bt),
                                             wt8[:, kk, :, :],
                                             start=False, stop=False,
                                             perf_mode=PM.DoubleRow)
                bsb2 = bpool.tile([2, 512], F16, tag="bias", name="bsb2")
                nc.sync.dma_start(bsb2[:], bod[:, n * 512:(n + 1) * 512])
                for bt in bts:
                    nc.tensor.matmul(ps1[bt][:], ones[:], bsb2[:],
                                     start=False, stop=True)
                for bt in bts:
                    osb = outpool.tile([128, 512], F32, tag="osb",
                                       name=f"osb{bt}")
                    nc.scalar.activation(osb[:], ps1[bt][:], AF.Copy,
                                         scale=ISCL)
                    nc.sync.dma_start(
                        out_d[bt * 128:(bt + 1) * 128,
                              n * 512:(n + 1) * 512],
                        osb[:])

    nc.compile()
    return nc


def _f16(a):
    return a.astype(np.float16)


def _e4m3(a):
    return a.astype(ml_dtypes.float8_e4m3fn)


def _split16(a):
    """fp32 -> (hi fp16, lo fp32)."""
    hi = a.astype(np.float16)
    return hi, a - hi.astype(np.float32)


def _prep_inputs(x, context, w1, b1, seg1, w2, b2, seg2, w_out, b_out):
    """Host-side splits + reshapes into partition-major DMA layouts."""
    c = np.ascontiguousarray

    def stat_pack(aT):
        """[d, BL] fp32 -> fp16 hi*2^12 [128, kt*BL] and fp8 pair
        [128, 2, kt*BL] (planes (lo*S, hi)), partition-major."""
        hi, lo = _split16(aT)
        kt = aT.shape[0] // 128
        his = (hi.astype(np.float32) * SCL).astype(np.float16)
        his = his.reshape(kt, 128, -1).transpose(1, 0, 2).reshape(128, -1)
        pair = np.empty((128, 2, kt * aT.shape[1]),
                        dtype=ml_dtypes.float8_e4m3fn)
        pair[:, 0, :] = _e4m3(lo * SCL).reshape(kt, 128, -1) \
            .transpose(1, 0, 2).reshape(128, -1)
        pair[:, 1, :] = _e4m3(hi.astype(np.float32)).reshape(kt, 128, -1) \
            .transpose(1, 0, 2).reshape(128, -1)
        return c(his), c(pair)

    def mov_pack(wT, kt, nch):
        """wT [d_in, d_out] fp32 -> fp16 hi [nch,kt//4,128,4,512] + fp8
        pair [nch,kt//4,128,4,2,512], partition-major contiguous lines."""
        hi, lo = _split16(wT)
        # [kt,128,nch,512] -> [nch, kt//4, 128, 4, 512]
        h4 = hi.reshape(kt // 4, 4, 128, nch, 512) \
            .transpose(3, 0, 2, 1, 4)
        pair = np.empty((nch, kt // 4, 128, 4, 2, 512),
                        dtype=ml_dtypes.float8_e4m3fn)
        pair[..., 0, :] = _e4m3(hi.astype(np.float32)) \
            .reshape(kt // 4, 4, 128, nch, 512).transpose(3, 0, 2, 1, 4)
        pair[..., 1, :] = _e4m3(lo * SCL) \
            .reshape(kt // 4, 4, 128, nch, 512).transpose(3, 0, 2, 1, 4)
        return c(h4), c(pair)

    def seg_pack(seg):
        segT = seg.reshape(HS, D_CTX).T         # [D_CTX, HS]
        hi, lo = _split16(segT)
        hi32 = hi.astype(np.float32)
        f16p = np.zeros((NCH + 1, 128, KT_IN, 512), dtype=np.float16)
        f8p = np.zeros((NCH + 1, 128, KT_IN, 2, 512),
                       dtype=ml_dtypes.float8_e4m3fn)
        h8 = _e4m3(hi32)
        l8 = _e4m3(lo * SCL)
        for cc in range(NCH + 1):
            w = CHW if cc < NCH else TAIL
            sl = slice(cc * CHW, cc * CHW + w)
            f16p[cc, :, :, :w] = hi[:, sl].reshape(KT_IN, 128, w) \
                .transpose(1, 0, 2)
            f8p[cc, :, :, 0, :w] = h8[:, sl].reshape(KT_IN, 128, w) \
                .transpose(1, 0, 2)
            f8p[cc, :, :, 1, :w] = l8[:, sl].reshape(KT_IN, 128, w) \
                .transpose(1, 0, 2)
        return c(f16p), c(f8p)

    def bias_pack(b, width):
        bs = b.astype(np.float32) * SCL
        hi = bs.astype(np.float16)
        lo = (bs - hi.astype(np.float32)).astype(np.float16)
        return c(np.stack([hi, lo]).reshape(2, width))

    w1t, w1t8 = mov_pack(w1.T, KT_IN, H // 512)
    w2t, w2t8 = mov_pack(w2.T, KT_H, H // 512)
    wot, wot8 = mov_pack(w_out.T, KT_H, D_OUT // 512)
    sg1, sg18 = seg_pack(seg1)
    sg2, sg28 = seg_pack(seg2)
    shared = {
        "w1t": w1t, "w1t8": w1t8, "w2t": w2t, "w2t8": w2t8,
        "wot": wot, "wot8": wot8,
        "sg1": sg1, "sg18": sg18, "sg2": sg2, "sg28": sg28,
        "b1d": bias_pack(b1, H), "b2d": bias_pack(b2, H),
        "bod": bias_pack(b_out, D_OUT),
    }
    in_maps = []
    for core in range(NCORES):
        sl = slice(core * BL, (core + 1) * BL)
        m = dict(shared)
        x1s, x8 = stat_pack(c(x[sl].T))
        c1s, c8 = stat_pack(c(context[sl].T))
        m["x1s"], m["x8"], m["c1s"], m["c8"] = x1s, x8, c1s, c8
        in_maps.append(m)
    return in_maps


_NC = None


def kernel(**inputs):
    global _NC
    if _NC is None:
        _NC = build_kernel()
    inputs = {k: np.ascontiguousarray(np.asarray(v), dtype=np.float32)
              for k, v in inputs.items()}
    in_maps = _prep_inputs(**inputs)
    res = run_bass_kernel_spmd(_NC, in_maps, list(range(NCORES)))
    return np.concatenate([res.results[i]["out"] for i in range(NCORES)],
                          axis=0)


# revision 13
# speedup vs baseline: 1.0864x; 1.0864x over previous
"""DendriticMLP Trainium2 kernel — 8-core batch-data-parallel.

v4: fp16 "hi" main pass + single fp8-e4m3 DoubleRow matmul fusing both
hi*lo cross terms (lo parts pre-scaled by 2^12), accumulating into one
fp32 PSUM at 2^12 scale (consumers fold 2^-12 back for free). On top of
v2:
  - layer-1 top-k bisection runs entirely on ACT (Sign+accum counts)
    and Pool (scalar updates), interleaved into the dend2 chunk loop,
    so DVE stays dedicated to the dend max/min PSUM reduces and the PE
    never stalls on PSUM-bank recycling.
  - bisection searches [0, M] (k-th winner is always positive here)
    instead of [-2M, 2M]: two bits tighter window, measurably lower
    top-k membership error.
  - the output-layer loop runs on b-tile PAIRS so w_out streams twice
    instead of four times (-16.8 MB DMA/core), and the two tail
    bisections of a pair run concurrently on DVE and ACT.
  - all weight/segment/stationary DRAM layouts are pre-packed
    partition-major on the host so every DMA line is 4-8 KB contiguous
    per partition (the v2 `rearrange` loads moved 1 KB lines).

Pipeline per core (B_local=512 rows, 4 b-tiles):
  y1 = x @ w1.T + b1
  dend1 = ctx @ seg1_flat.T (41 chunks) -> strided max/min reduce over
      10 segments (DVE) -> sel = where(mx+mn>0, mx, mn) ->
      gate = sigmoid(2^-12 sel) (ACT)
  g = y1*gate; exact top-k (k=102) via 20-iteration count bisection
  h split to fp16 hi (+2^12-scaled lo), transposed on PE, fp8 recast
  layer 2 same; out = h2 @ w_out.T + b_out.
"""
import numpy as np
from contextlib import ExitStack

import concourse.bass as bass
import concourse.tile as tile
from concourse import bacc, mybir, masks
from concourse.bass_utils import run_bass_kernel_spmd
import ml_dtypes

F32 = mybir.dt.float32
F16 = mybir.dt.float16
F8 = mybir.dt.float8e4
AF = mybir.ActivationFunctionType
OP = mybir.AluOpType
AX = mybir.AxisListType
PM = mybir.MatmulPerfMode

# problem dims (hardcoded per contract)
B, D_IN, H, S, D_CTX, D_OUT = 4096, 1024, 2048, 10, 1024, 1024
KW = 102                 # k-winners per row
NCORES = 8
BL = B // NCORES         # 512 rows per core
BT = BL // 128           # 4 b-tiles of 128 rows
NITER = 20               # bisection iterations

HS = H * S               # 20480
CHW = 510                # dend chunk width (51 groups of 10)
NCH = HS // CHW          # 40 full chunks
TAIL = HS - NCH * CHW    # 80 (8 groups)
GR = CHW // S            # 51 groups per chunk
KT_IN = D_IN // 128      # 8 contraction tiles for d=1024
KT_H = H // 128          # 16 contraction tiles for d=2048

SCL = 4096.0             # 2^12 psum scale
ISCL = 1.0 / SCL


def build_kernel(loop_n=None, weights_internal=False):
    """weights_internal=True: big replicated weights become Internal DRAM
    (uninitialized) so timing runs skip the host transfer; the on-device
    DMA and compute per iteration are identical."""
    nc = bacc.Bacc("TRN2", target_bir_lowering=False, debug=False,
                   num_devices=NCORES)

    def din(name, shape, dt=F32):
        return nc.dram_tensor(name, shape, dt, kind="ExternalInput").ap()

    if weights_internal:
        _din_small = din

        def din(name, shape, dt=F32):  # noqa: F811
            if name in ("x1s", "x8", "c1s", "c8"):
                return _din_small(name, shape, dt)
            return nc.dram_tensor(name, shape, dt, kind="Internal").ap()

    # activations, pre-transposed partition-major on the host:
    # fp16 hi (stationary side pre-scaled 2^12) + fp8 plane pairs
    x1s_d = din("x1s", [128, KT_IN * BL], F16)
    x8_d = din("x8", [128, 2, KT_IN * BL], F8)     # planes (x2*S, x1)
    c1s_d = din("c1s", [128, KT_IN * BL], F16)
    c8_d = din("c8", [128, 2, KT_IN * BL], F8)
    # dense-layer weights: [chunk, qgroup, 128, 4(k), ...] contiguous lines
    w1t_d = din("w1t", [H // 512, KT_IN // 4, 128, 4, 512], F16)
    w1t8_d = din("w1t8", [H // 512, KT_IN // 4, 128, 4, 2, 512], F8)
    w2t_d = din("w2t", [H // 512, KT_H // 4, 128, 4, 512], F16)
    w2t8_d = din("w2t8", [H // 512, KT_H // 4, 128, 4, 2, 512], F8)
    wot_d = din("wot", [D_OUT // 512, KT_H // 4, 128, 4, 512], F16)
    wot8_d = din("wot8", [D_OUT // 512, KT_H // 4, 128, 4, 2, 512], F8)
    # dendrite segments, 41 uniform padded chunks, partition-major
    sg1_d = din("sg1", [NCH + 1, 128, KT_IN, 512], F16)
    sg18_d = din("sg18", [NCH + 1, 128, KT_IN, 2, 512], F8)
    sg2_d = din("sg2", [NCH + 1, 128, KT_IN, 512], F16)
    sg28_d = din("sg28", [NCH + 1, 128, KT_IN, 2, 512], F8)
    # biases pre-scaled by 2^12, fp16 hi/lo rows
    b1d = din("b1d", [2, H], F16)
    b2d = din("b2d", [2, H], F16)
    bod = din("bod", [2, D_OUT], F16)
    out_d = nc.dram_tensor("out", [BL, D_OUT], F32, kind="ExternalOutput").ap()

    with tile.TileContext(nc) as tc, ExitStack() as ctx:
        if loop_n is not None:
            ctx.enter_context(tc.For_i(0, loop_n, 1))
        cpool = ctx.enter_context(tc.tile_pool(name="const", bufs=1))
        apool = ctx.enter_context(tc.tile_pool(name="acts", bufs=1))
        ypool = ctx.enter_context(tc.tile_pool(name="y", bufs=BT))
        selpool = ctx.enter_context(tc.tile_pool(name="sel", bufs=BT))
        mnpool = ctx.enter_context(tc.tile_pool(name="mn", bufs=4))
        wpool = ctx.enter_context(tc.tile_pool(name="w", bufs=4))
        spool = ctx.enter_context(tc.tile_pool(name="seg", bufs=2))
        outpool = ctx.enter_context(tc.tile_pool(name="osb", bufs=1))
        tinypool = ctx.enter_context(tc.tile_pool(name="tiny", bufs=1))
        hpool = ctx.enter_context(tc.tile_pool(name="hsplit", bufs=1))
        psy = ctx.enter_context(tc.tile_pool(name="psy", bufs=BT, space="PSUM"))
        psd = ctx.enter_context(tc.tile_pool(name="psd", bufs=2, space="PSUM"))
        pst = ctx.enter_context(tc.tile_pool(name="pst", bufs=2, space="PSUM"))

        # constants
        ident16 = cpool.tile([128, 128], F16)
        masks.make_identity(nc, ident16[:])
        ones = cpool.tile([2, 128], F16)
        nc.gpsimd.memset(ones[:], 1.0)
        cnt_bias = cpool.tile([128, 1], F32)
        nc.gpsimd.memset(cnt_bias[:], float(H - 2 * KW) + 0.5)
        bpool = ctx.enter_context(tc.tile_pool(name="bias", bufs=2))

        # ctx stationary (lives through both dend phases)
        c1s_sb = apool.tile([128, KT_IN * BL], F16)
        c8_sb = apool.tile([128, 2, KT_IN * BL], F8)
        nc.sync.dma_start(c1s_sb[:], c1s_d)
        nc.scalar.dma_start(c8_sb[:], c8_d)
        # x stationary (dead after y1) shares slots with the h pair below
        x1s_sb = apool.tile([128, KT_H * 512], F16, tag="stat16", name="x1s_sb")
        x8_sb = apool.tile([128, 2, KT_H * 512], F8, tag="stat8", name="x8_sb")
        nc.sync.dma_start(x1s_sb[:, :KT_IN * BL], x1s_d)
        nc.scalar.dma_start(x8_sb[:, :, :KT_IN * BL], x8_d)

        def st16(sb, kk, bt):
            return sb[:, kk * BL + bt * 128: kk * BL + (bt + 1) * 128]

        def st8(sb, kk, bt):
            return sb[:, :, kk * BL + bt * 128: kk * BL + (bt + 1) * 128]

        # per-bt tiny state: cols 0=M 1=lo 2=w 3=t 4=pred 5=cnt 6=tneg
        tiny = [tinypool.tile([128, 8], F32, tag=f"tiny{bt}", name=f"tiny{bt}")
                for bt in range(BT)]

        scr_cell = []

        def get_scr():
            if not scr_cell:
                scr_cell.append(apool.tile([128, H], F32, tag="scr",
                                           name="scr"))
            return scr_cell[0]

        act_scr = apool.tile([128, H], F32, tag="ascr", name="act_scr")

        def yphase(s16, s8, kt, w16_dram, w8_dram, bias_dram, width, lay,
                   bts=None, ytiles=None):
            """Dense y = act @ W.T (+bias), planB, for the given b-tiles."""
            if bts is None:
                bts = tuple(range(BT))
            if ytiles is None:
                ytiles = [ypool.tile([128, H], F32, tag="y",
                                     name=f"y{lay}_{i}") for i in range(BT)]
            nch = width // 512
            for n in range(nch):
                ps = {bt: psy.tile([128, 512], F32, tag="psy",
                                   name=f"psy{bt}") for bt in bts}
                for q in range(kt // 4):
                    wt = wpool.tile([128, 4, 512], F16, tag="w")
                    nc.sync.dma_start(wt[:], w16_dram[n, q])
                    wt8 = wpool.tile([128, 4, 2, 512], F8, tag="w8")
                    nc.scalar.dma_start(wt8[:], w8_dram[n, q])
                    for kk in range(4):
                        k = q * 4 + kk
                        for bt in bts:
                            nc.tensor.matmul(ps[bt][:], s16(k, bt),
                                             wt[:, kk, :],
                                             start=(k == 0), stop=False)
                            nc.tensor.matmul(ps[bt][:], s8(k, bt),
                                             wt8[:, kk, :, :],
                                             start=False, stop=False,
                                             perf_mode=PM.DoubleRow)
                bsb = bpool.tile([2, 512], F16, tag="bias", name="bsb")
                nc.sync.dma_start(bsb[:], bias_dram[:, n * 512:(n + 1) * 512])
                for bt in bts:
                    nc.tensor.matmul(ps[bt][:], ones[:], bsb[:],
                                     start=False, stop=True)
                for bt in bts:
                    nc.scalar.activation(ytiles[bt][:, n * 512:(n + 1) * 512],
                                         ps[bt][:], AF.Copy, scale=ISCL)
            return ytiles

        def dendphase(sg16, sg8, seltiles, filler=None):
            """dend matmuls + per-chunk absmax-signed sel on DVE:
            sel-slice = where(mx+mn>0, mx, mn) right after the max/min
            reduces. filler() interleaves ACT/Pool bisection steps."""
            for c in range(NCH + 1):
                w = CHW if c < NCH else TAIL
                g = GR if c < NCH else TAIL // S
                sgt = spool.tile([128, KT_IN, 512], F16, tag="seg", name="sgt")
                nc.sync.dma_start(sgt[:], sg16[c])
                sgt8 = spool.tile([128, KT_IN, 2, 512], F8, tag="seg8",
                                  name="sgt8")
                nc.scalar.dma_start(sgt8[:], sg8[c])
                for bt in range(BT):
                    pd = psd.tile([128, 512], F32, tag="psd")
                    for k in range(KT_IN):
                        nc.tensor.matmul(pd[:, :w], st16(c1s_sb, k, bt),
                                         sgt[:, k, :w],
                                         start=(k == 0), stop=False)
                    for k in range(KT_IN):
                        nc.tensor.matmul(pd[:, :w], st8(c8_sb, k, bt),
                                         sgt8[:, k, :, :w],
                                         start=False, stop=(k == KT_IN - 1),
                                         perf_mode=PM.DoubleRow)
                    view = pd[:, :w].rearrange("p (g s) -> p g s", s=S)
                    sl = seltiles[bt][:, c * GR:c * GR + g]
                    mnt = mnpool.tile([128, 64], F32, tag="mnt", name="mnt")
                    prt = mnpool.tile([128, 64], F32, tag="prt", name="prt")
                    nc.vector.tensor_reduce(sl, view, axis=AX.X, op=OP.max)
                    nc.vector.tensor_reduce(mnt[:, :g], view, axis=AX.X,
                                            op=OP.min)
                    nc.vector.tensor_add(prt[:, :g], sl, mnt[:, :g])
                    nc.vector.tensor_scalar(prt[:, :g], prt[:, :g], 0.0, None,
                                            op0=OP.is_le)
                    nc.vector.copy_predicated(
                        sl, prt[:, :g].bitcast(mybir.dt.int32), mnt[:, :g])
                if filler is not None:
                    filler(3)

        def selgate(seltiles):
            """gate = sigmoid(2^-12 * sel) in place."""
            for bt in range(BT):
                nc.scalar.activation(seltiles[bt][:], seltiles[bt][:],
                                     AF.Sigmoid, scale=ISCL)

        def gate_mul(ytiles, seltiles, bts=None):
            """g = y*gate in place on y tile; M = absmax(g) into tiny col 0."""
            for bt in (range(BT) if bts is None else bts):
                nc.vector.tensor_mul(ytiles[bt][:], ytiles[bt][:],
                                     seltiles[bt][:])
                nc.vector.tensor_reduce(tiny[bt][:, 0:1], ytiles[bt][:],
                                        axis=AX.X, op=OP.max,
                                        apply_absolute_value=True)

        def bisect_steps(ytile, bt):
            """Generator: one bisection iteration per yield, ENTIRELY on ACT
            (midpoint form): probe mu, half-step s; mu += sign*s each iter.
            Only the absmax init (DVE) and final mask (DVE) cross engines,
            so dend reduces never wait behind bisection.
            Search [0, 1.001*M]: the 102nd of 2048 is always > 0 here."""
            t = tiny[bt]
            M, mu, sp, tt_, sg, cnt = (t[:, i:i + 1] for i in range(6))
            tneg = t[:, 6:7]
            nc.scalar.activation(mu, M, AF.Copy, scale=0.5005, bias=2.5e-31)
            nc.scalar.activation(sp, M, AF.Copy, scale=0.25025,
                                 bias=1.25e-31)
            yield
            for it in range(NITER):
                nc.scalar.activation(tneg, mu, AF.Copy, scale=-1.0)
                # sum(sign(g - mu)); count>=KW  <=>  sum >= 2*KW - H
                nc.scalar.activation(act_scr[:], ytile[:], AF.Sign,
                                     bias=tneg, accum_out=cnt)
                nc.scalar.activation(sg, cnt, AF.Sign, bias=cnt_bias[:])
                nc.scalar.activation(mu, sg, AF.Identity, scale=sp, bias=mu)
                nc.scalar.activation(sp, sp, AF.Copy, scale=0.5)
                yield
            # lower bound of final bracket: lo = mu - 2*s
            nc.scalar.activation(tt_, sp, AF.Identity, scale=-2.0, bias=mu)
            nc.vector.scalar_tensor_tensor(ytile[:], ytile[:], tt_, ytile[:],
                                           op0=OP.is_ge, op1=OP.mult)

        def make_filler(gens):
            """Round-robin advance the chain generators n steps per call."""
            alive = list(gens)

            def fill(n):
                for _ in range(n):
                    if not alive:
                        return
                    g = alive.pop(0)
                    try:
                        next(g)
                        alive.append(g)
                    except StopIteration:
                        pass
            fill.drain = lambda: fill(10**6)
            return fill

        def bisect_chain(ytile, bt, on_act):
            """One b-tile's top-k bisection + in-place mask. The whole chain
            runs on ONE engine (ACT midpoint form, or DVE lo/w form) so no
            per-iteration cross-engine semaphores exist."""
            t = tiny[bt]
            M, lo, w, tt_, pred, cnt = (t[:, i:i + 1] for i in range(6))
            tneg = t[:, 6:7]
            if on_act:
                mu, sp, sg = lo, w, pred
                nc.scalar.activation(mu, M, AF.Copy, scale=0.5005,
                                     bias=2.5e-31)
                nc.scalar.activation(sp, M, AF.Copy, scale=0.25025,
                                     bias=1.25e-31)
                for it in range(NITER):
                    nc.scalar.activation(tneg, mu, AF.Copy, scale=-1.0)
                    nc.scalar.activation(act_scr[:], ytile[:], AF.Sign,
                                         bias=tneg, accum_out=cnt)
                    nc.scalar.activation(sg, cnt, AF.Sign, bias=cnt_bias[:])
                    nc.scalar.activation(mu, sg, AF.Identity, scale=sp,
                                         bias=mu)
                    nc.scalar.activation(sp, sp, AF.Copy, scale=0.5)
                nc.scalar.activation(tt_, sp, AF.Identity, scale=-2.0,
                                     bias=mu)
            else:
                ve = nc.vector
                ve.tensor_scalar(w, M, 1.001, 1e-30, op0=OP.mult, op1=OP.add)
                ve.tensor_scalar_mul(lo, M, 0.0)
                for it in range(NITER):
                    ve.tensor_scalar_mul(w, w, 0.5)
                    ve.tensor_add(tt_, lo, w)
                    ve.scalar_tensor_tensor(
                        get_scr()[:], ytile[:], tt_, ytile[:],
                        op0=OP.is_ge, op1=OP.bypass, accum_out=cnt)
                    ve.tensor_scalar(pred, cnt, float(KW), None,
                                     op0=OP.is_ge)
                    ve.scalar_tensor_tensor(lo, pred, w, lo,
                                            op0=OP.mult, op1=OP.add)
                tt_ = lo
            nc.vector.scalar_tensor_tensor(ytile[:], ytile[:], tt_, ytile[:],
                                           op0=OP.is_ge, op1=OP.mult)

        # h split + transpose: produce fp16-scaled hi stationary + fp8 pair
        h1T = apool.tile([128, KT_H * 512], F16, tag="stat16", name="h1T")
        h8T = apool.tile([128, 2, KT_H * 512], F8, tag="stat8", name="h8T")

        def transpose_bt(ytile, bt):
            """ytile holds masked h (fp32). Writes h1T/h8T bt-columns."""
            h1 = hpool.tile([128, H], F16, tag="h1")
            nc.scalar.activation(h1[:], ytile[:], AF.Copy)
            h2s = hpool.tile([128, H], F16, tag="h2s")
            nc.vector.scalar_tensor_tensor(get_scr()[:], h1[:], -1.0,
                                           ytile[:], op0=OP.mult, op1=OP.add)
            nc.vector.tensor_scalar_mul(h2s[:], get_scr()[:], SCL)
            for kb in range(KT_H):
                dst = slice(kb * 512 + bt * 128, kb * 512 + (bt + 1) * 128)
                p1 = pst.tile([128, 128], F16, tag="pst", name="p1")
                nc.tensor.transpose(p1[:], h1[:, kb * 128:(kb + 1) * 128],
                                    ident16[:])
                nc.scalar.activation(h1T[:, dst], p1[:], AF.Copy, scale=SCL)
                nc.scalar.activation(h8T[:, 1, dst], p1[:], AF.Copy)
                p2 = pst.tile([128, 128], F16, tag="pst", name="p2")
                nc.tensor.transpose(p2[:], h2s[:, kb * 128:(kb + 1) * 128],
                                    ident16[:])
                nc.scalar.activation(h8T[:, 0, dst], p2[:], AF.Copy)

        def h_st16(k, bt):
            return h1T[:, k * 512 + bt * 128: k * 512 + (bt + 1) * 128]

        def h_st8(k, bt):
            return h8T[:, :, k * 512 + bt * 128: k * 512 + (bt + 1) * 128]

        # ---------------- layer 1 ----------------
        sel1 = [selpool.tile([128, H], F32, tag="sel", name=f"sel1_{i}")
                for i in range(BT)]
        y1 = yphase(lambda k, bt: st16(x1s_sb, k, bt),
                    lambda k, bt: st8(x8_sb, k, bt),
                    KT_IN, w1t_d, w1t8_d, b1d, H, lay=0)
        dendphase(sg1_d, sg18_d, sel1)
        selgate(sel1)
        gate_mul(y1, sel1)

        # dend2 issued with bisect1 interleaved as ACT/Pool filler steps:
        # DVE does only dend reduces, PE never waits on bisect1.
        sel2 = [selpool.tile([128, H], F32, tag="sel", name=f"sel2_{i}")
                for i in range(BT)]
        fill1 = make_filler([bisect_steps(y1[bt], bt) for bt in range(BT)])
        dendphase(sg2_d, sg28_d, sel2, filler=fill1)
        fill1.drain()
        selgate(sel2)

        for bt in range(BT):
            transpose_bt(y1[bt], bt)

        # ---------------- layer 2 ----------------
        y2 = yphase(h_st16, h_st8, KT_H, w2t_d, w2t8_d, b2d, H, lay=1)
        gate_mul(y2, sel2)

        # tail on b-tile pairs: both bisections of a pair run concurrently
        # (one on DVE, one on ACT), then the pair shares one w_out stream
        # (2x instead of 4x w_out DMA).
        for pr in range(BT // 2):
            bts = (2 * pr, 2 * pr + 1)
            for bt in bts:
                bisect_chain(y2[bt], bt, on_act=bool(bt % 2))
            for bt in bts:
                transpose_bt(y2[bt], bt)
            for n in range(D_OUT // 512):
                ps1 = {bt: psy.tile([128, 512], F32, tag="psy",
                                    name=f"pso{bt}") for bt in bts}
                for q in range(KT_H // 4):
                    wt = wpool.tile([128, 4, 512], F16, tag="w")
                    nc.sync.dma_start(wt[:], wot_d[n, q])
                    wt8 = wpool.tile([128, 4, 2, 512], F8, tag="w8")
                    nc.scalar.dma_start(wt8[:], wot8_d[n, q])
                    for kk in range(4):
                        k = q * 4 + kk
                        for bt in bts:
                            nc.tensor.matmul(ps1[bt][:], h_st16(k, bt),
                                             wt[:, kk, :],
                                             start=(k == 0), stop=False)
                            nc.tensor.matmul(ps1[bt][:], h_st8(k, bt),
                                             wt8[:, kk, :, :],
                                             start=False, stop=False,
                                             perf_mode=PM.DoubleRow)
                bsb2 = bpool.tile([2, 512], F16, tag="bias", name="bsb2")
                nc.sync.dma_start(bsb2[:], bod[:, n * 512:(n + 1) * 512])
                for bt in bts:
                    nc.tensor.matmul(ps1[bt][:], ones[:], bsb2[:],
                                     start=False, stop=True)
                for bt in bts:
                    osb = outpool.tile([128, 512], F32, tag="osb",
                                       name=f"osb{bt}")
                    nc.scalar.activation(osb[:], ps1[bt][:], AF.Copy,
                                         scale=ISCL)
                    nc.sync.dma_start(
                        out_d[bt * 128:(bt + 1) * 128,
                              n * 512:(n + 1) * 512],
                        osb[:])

    nc.compile()
    return nc


def _f16(a):
    return a.astype(np.float16)


def _e4m3(a):
    return a.astype(ml_dtypes.float8_e4m3fn)


def _split16(a):
    """fp32 -> (hi fp16, lo fp32)."""
    hi = a.astype(np.float16)
    return hi, a - hi.astype(np.float32)


def _prep_inputs(x, context, w1, b1, seg1, w2, b2, seg2, w_out, b_out):
    """Host-side splits + reshapes into partition-major DMA layouts."""
    c = np.ascontiguousarray

    def stat_pack(aT):
        """[d, BL] fp32 -> fp16 hi*2^12 [128, kt*BL] and fp8 pair
        [128, 2, kt*BL] (planes (lo*S, hi)), partition-major."""
        hi, lo = _split16(aT)
        kt = aT.shape[0] // 128
        his = (hi.astype(np.float32) * SCL).astype(np.float16)
        his = his.reshape(kt, 128, -1).transpose(1, 0, 2).reshape(128, -1)
        pair = np.empty((128, 2, kt * aT.shape[1]),
                        dtype=ml_dtypes.float8_e4m3fn)
        pair[:, 0, :] = _e4m3(lo * SCL).reshape(kt, 128, -1) \
            .transpose(1, 0, 2).reshape(128, -1)
        pair[:, 1, :] = _e4m3(hi.astype(np.float32)).reshape(kt, 128, -1) \
            .transpose(1, 0, 2).reshape(128, -1)
        return c(his), c(pair)

    def mov_pack(wT, kt, nch):
        """wT [d_in, d_out] fp32 -> fp16 hi [nch,kt//4,128,4,512] + fp8
        pair [nch,kt//4,128,4,2,512], partition-major contiguous lines."""
        hi, lo = _split16(wT)
        # [kt,128,nch,512] -> [nch, kt//4, 128, 4, 512]
        h4 = hi.reshape(kt // 4, 4, 128, nch, 512) \
            .transpose(3, 0, 2, 1, 4)
        pair = np.empty((nch, kt // 4, 128, 4, 2, 512),
                        dtype=ml_dtypes.float8_e4m3fn)
        pair[..., 0, :] = _e4m3(hi.astype(np.float32)) \
            .reshape(kt // 4, 4, 128, nch, 512).transpose(3, 0, 2, 1, 4)
        pair[..., 1, :] = _e4m3(lo * SCL) \
            .reshape(kt // 4, 4, 128, nch, 512).transpose(3, 0, 2, 1, 4)
        return c(h4), c(pair)

    def seg_pack(seg):
        segT = seg.reshape(HS, D_CTX).T         # [D_CTX, HS]
        hi, lo = _split16(segT)
        hi32 = hi.astype(np.float32)
        f16p = np.zeros((NCH + 1, 128, KT_IN, 512), dtype=np.float16)
        f8p = np.zeros((NCH + 1, 128, KT_IN, 2, 512),
                       dtype=ml_dtypes.float8_e4m3fn)
        h8 = _e4m3(hi32)
        l8 = _e4m3(lo * SCL)
        for cc in range(NCH + 1):
            w = CHW if cc < NCH else TAIL
            sl = slice(cc * CHW, cc * CHW + w)
            f16p[cc, :, :, :w] = hi[:, sl].reshape(KT_IN, 128, w) \
                .transpose(1, 0, 2)
            f8p[cc, :, :, 0, :w] = h8[:, sl].reshape(KT_IN, 128, w) \
                .transpose(1, 0, 2)
            f8p[cc, :, :, 1, :w] = l8[:, sl].reshape(KT_IN, 128, w) \
                .transpose(1, 0, 2)
        return c(f16p), c(f8p)

    def bias_pack(b, width):
        bs = b.astype(np.float32) * SCL
        hi = bs.astype(np.float16)
        lo = (bs - hi.astype(np.float32)).astype(np.float16)
        return c(np.stack([hi, lo]).reshape(2, width))

    w1t, w1t8 = mov_pack(w1.T, KT_IN, H // 512)
    w2t, w2t8 = mov_pack(w2.T, KT_H, H // 512)
    wot, wot8 = mov_pack(w_out.T, KT_H, D_OUT // 512)
    sg1, sg18 = seg_pack(seg1)
    sg2, sg28 = seg_pack(seg2)
    shared = {
        "w1t": w1t, "w1t8": w1t8, "w2t": w2t, "w2t8": w2t8,
        "wot": wot, "wot8": wot8,
        "sg1": sg1, "sg18": sg18, "sg2": sg2, "sg28": sg28,
        "b1d": bias_pack(b1, H), "b2d": bias_pack(b2, H),
        "bod": bias_pack(b_out, D_OUT),
    }
    in_maps = []
    for core in range(NCORES):
        sl = slice(core * BL, (core + 1) * BL)
        m = dict(shared)
        x1s, x8 = stat_pack(c(x[sl].T))
        c1s, c8 = stat_pack(c(context[sl].T))
        m["x1s"], m["x8"], m["c1s"], m["c8"] = x1s, x8, c1s, c8
        in_maps.append(m)
    return in_maps


_NC = None


def kernel(**inputs):
    global _NC
    if _NC is None:
        _NC = build_kernel()
    inputs = {k: np.ascontiguousarray(np.asarray(v), dtype=np.float32)
              for k, v in inputs.items()}
    in_maps = _prep_inputs(**inputs)
    res = run_bass_kernel_spmd(_NC, in_maps, list(range(NCORES)))
    return np.concatenate([res.results[i]["out"] for i in range(NCORES)],
                          axis=0)
